# revision 8
# baseline (speedup 1.0000x reference)
"""Causal self-attention (GQA + RoPE + QK-RMSNorm) on 8 trn2 NeuronCores.

Reference (B=2, T=2048, C=2048, 16 q-heads / 4 kv-heads, head_dim 128):
    q = rms_norm(rope(x @ Wq)) / sqrt(128); k = rms_norm(rope(x @ Wk))
    att = softmax_causal(q k^T / sqrt(128)); y = (att @ v) @ Wp

Sharding: core = 4*b + g (b = batch, g = head-group/quarter).
Each core projects+attends q-heads 4g..4g+3 (kv-head g) over the full
causal sequence of its batch, computes the partial output projection
y_local @ Wp[local-head rows, :] over all T, and a ReduceScatter(add)
per 512-t-chunk sums the 4 partials and shards c_out: core g keeps
c_out block g.  Host places disjoint shards only.

Perf structure:
 - Q/K projections run in fp8 (e4m3) DoubleRow mode (2x PE rate).  The
   QK rms-norm makes any weight scale drop out, so Wq/Wk are pre-scaled
   64x into fp8 range; V and Wp stay bf16 for accuracy.
 - All rms-norms via Ln+Exp on the scalar engine: the
   natural_log_exp activation table serves phase-2 Exp too, so the
   whole kernel uses ONE act table (no reloads when phases overlap).
 - Attention S^T layout: exp(scale*s) with per-partition 1/rms_k scale;
   causal handled block-wise with column-trimmed diagonal blocks and a
   single [128,512] (j>=p) mask.
 - Everything is emitted tqc-outer so phase 1 (proj) -> phase 2 (attn)
   -> phase 4 (out-proj) -> RS pipeline per 512-t chunk.
"""

import math

import ml_dtypes
import numpy as np

B, T, C = 2, 2048, 2048
NH, NKV, HD = 16, 4, 128
G = 4  # q-heads per core
EPS = 1e-6
NCB = C // 128  # 16 contraction blocks
NPAIR = NCB // 2  # 8 fp8 DoubleRow pairs
NTCH = T // 512  # 4 t-chunks
NTKB = T // 128  # 16 key blocks
LOG_HD = math.log(HD)

_CACHE = {}


def _build():
    import concourse.mybir as mybir
    import concourse.tile as tile
    from concourse import bacc
    from concourse.masks import make_identity
    from contextlib import ExitStack

    F32 = mybir.dt.float32
    BF16 = mybir.dt.bfloat16
    F8 = mybir.dt.float8e4
    AF = mybir.ActivationFunctionType
    DR = mybir.MatmulPerfMode.DoubleRow

    nc = bacc.Bacc(None, target_bir_lowering=False, num_devices=8)

    x8 = nc.dram_tensor("x8", [128, NPAIR, 2, T], F8, kind="ExternalInput")
    xb = nc.dram_tensor("xb", [128, NCB, T], BF16, kind="ExternalInput")
    wq8 = nc.dram_tensor("wq8", [128, NPAIR, 2, G * HD], F8, kind="ExternalInput")
    wk8 = nc.dram_tensor("wk8", [128, NPAIR, 2, HD], F8, kind="ExternalInput")
    wv = nc.dram_tensor("wv", [128, NCB, HD], BF16, kind="ExternalInput")
    wp = nc.dram_tensor("wp", [128, G, C], BF16, kind="ExternalInput")
    cos2 = nc.dram_tensor("cos2", [128, T], BF16, kind="ExternalInput")
    sin2 = nc.dram_tensor("sin2", [128, T], BF16, kind="ExternalInput")
    maskin = nc.dram_tensor("maskin", [128, 512], BF16, kind="ExternalInput")
    outT = nc.dram_tensor("outT", [NTCH, 512, 512], BF16, kind="ExternalOutput")

    groups = [[0, 1, 2, 3], [4, 5, 6, 7]]

    with tile.TileContext(nc) as tc:
        with ExitStack() as es:
            dram = es.enter_context(tc.tile_pool(name="dram", bufs=2, space="DRAM"))

            consts = es.enter_context(tc.tile_pool(name="consts", bufs=1))
            ones_bf = consts.tile([128, 1], BF16)
            nc.vector.memset(ones_bf[:], 1.0)
            eps_c = consts.tile([128, 1], F32)
            nc.vector.memset(eps_c[:], EPS)
            zero_c = consts.tile([128, 1], F32)
            nc.vector.memset(zero_c[:], 0.0)
            nlhd_c = consts.tile([128, 1], F32)
            nc.vector.memset(nlhd_c[:], -LOG_HD)
            ident_bf = consts.tile([128, 128], BF16)
            make_identity(nc, ident_bf[:])
            cos_sb = consts.tile([128, T], BF16)
            sin_sb = consts.tile([128, T], BF16)
            mask_sb = consts.tile([128, 512], BF16)
            nc.sync.dma_start(out=cos_sb[:], in_=cos2[:])
            nc.sync.dma_start(out=sin_sb[:], in_=sin2[:])
            nc.sync.dma_start(out=mask_sb[:], in_=maskin[:])

            wpool = es.enter_context(tc.tile_pool(name="w", bufs=1))
            wq_sb = wpool.tile([128, NPAIR, 2, G * HD], F8)
            wk_sb = wpool.tile([128, NPAIR, 2, HD], F8)
            wv_sb = wpool.tile([128, NCB, HD], BF16)
            wp_sb = wpool.tile([128, G, C], BF16)
            nc.sync.dma_start(out=wk_sb[:], in_=wk8[:])
            nc.sync.dma_start(out=wq_sb[:], in_=wq8[:])
            nc.sync.dma_start(out=wv_sb[:], in_=wv[:])
            nc.sync.dma_start(out=wp_sb[:], in_=wp[:])

            acts = es.enter_context(tc.tile_pool(name="acts", bufs=1))
            qT_sb = acts.tile([128, G, T], BF16)
            kT_sb = acts.tile([128, T], BF16)
            v_sb = acts.tile([128, NTKB, HD], BF16)
            yT_sb = acts.tile([128, G, T], BF16)
            rk_col = acts.tile([128, NTKB], F32)

            xs = es.enter_context(tc.tile_pool(name="xs", bufs=2))
            tmp = es.enter_context(tc.tile_pool(name="tmp", bufs=3))
            qro = es.enter_context(tc.tile_pool(name="qro", bufs=6))
            sml = es.enter_context(tc.tile_pool(name="sml", bufs=3))
            pts = es.enter_context(tc.tile_pool(name="pts", bufs=5))
            pos = es.enter_context(tc.tile_pool(name="pos", bufs=3))
            ps = es.enter_context(tc.tile_pool(name="ps", bufs=8, space="PSUM"))

            def rope(dst, src_psum, tcs, w=512):
                """dst = src*cos + rotate_half(src)*sin over t-cols tcs."""
                rot = tmp.tile([128, 512], BF16, tag="rot")
                nc.scalar.copy(out=rot[0:64, 0:w], in_=src_psum[64:128, 0:w])
                nc.scalar.copy(out=rot[64:128, 0:w], in_=src_psum[0:64, 0:w])
                qr = tmp.tile([128, 512], F32, tag="qr")
                nc.vector.tensor_mul(qr[:, 0:w], src_psum[:, 0:w], cos_sb[:, tcs])
                nc.vector.tensor_mul(rot[:, 0:w], rot[:, 0:w], sin_sb[:, tcs])
                nc.vector.tensor_add(dst, qr[:, 0:w], rot[:, 0:w])

            for tch in range(NTCH):
                tcs = slice(512 * tch, 512 * tch + 512)

                x8_t = xs.tile([128, NPAIR, 2, 512], F8, tag="x8")
                nc.sync.dma_start(out=x8_t[:], in_=x8[:, :, :, tcs])
                xb_t = xs.tile([128, NCB, 512], BF16, tag="xb")
                nc.sync.dma_start(out=xb_t[:], in_=xb[:, :, tcs])

                # ---- K projection (fp8 DoubleRow) + rope + 1/rms ----
                ps_k = ps.tile([128, 512], F32, tag="ps")
                for j in range(NPAIR):
                    nc.tensor.matmul(
                        ps_k[:], wk_sb[:, j], x8_t[:, j],
                        start=(j == 0), stop=(j == NPAIR - 1), perf_mode=DR,
                    )
                rope(kT_sb[:, tcs], ps_k[:], tcs)
                ksq = sml.tile([128, 512], BF16, tag="ksq")
                nc.vector.tensor_mul(ksq[:], kT_sb[:, tcs], kT_sb[:, tcs])
                ps_kc = ps.tile([128, 4], F32, tag="ps")
                for jj in range(4):
                    nc.tensor.matmul(
                        ps_kc[:, jj : jj + 1],
                        ksq[:, 128 * jj : 128 * jj + 128],
                        ones_bf[:],
                        start=True, stop=True,
                    )
                u_k = sml.tile([128, 4], F32, tag="uk")
                nc.scalar.activation(
                    out=u_k[:], in_=ps_kc[:], func=AF.Ln, scale=1.0 / HD, bias=eps_c[:]
                )
                nc.scalar.activation(
                    out=rk_col[:, 4 * tch : 4 * tch + 4], in_=u_k[:],
                    func=AF.Exp, scale=-0.5, bias=zero_c[:],
                )

                # ---- V projection (bf16) + PE transpose into v_sb ----
                ps_v = ps.tile([128, 512], F32, tag="ps")
                for cb in range(NCB):
                    nc.tensor.matmul(
                        ps_v[:], wv_sb[:, cb], xb_t[:, cb],
                        start=(cb == 0), stop=(cb == NCB - 1),
                    )
                vbf = sml.tile([128, 512], BF16, tag="vbf")
                nc.scalar.copy(out=vbf[:], in_=ps_v[:])
                for tt in range(4):
                    ps_tr = ps.tile([128, 128], BF16, tag="ps")
                    nc.tensor.transpose(
                        ps_tr[:], vbf[:, 128 * tt : 128 * tt + 128], ident_bf[:]
                    )
                    nc.vector.tensor_copy(out=v_sb[:, 4 * tch + tt, :], in_=ps_tr[:])

                # ---- Q projections (fp8 DoubleRow) + rope + sumsq ----
                # Per-query 1/(HD*rms_q) in column layout: ps_qc col 4h+jj
                # holds norms of head h, 128-t-block jj.
                qropes = []
                ps_qc = ps.tile([128, 16], F32, tag="ps")
                for h in range(G):
                    ps_q = ps.tile([128, 512], F32, tag="ps")
                    for j in range(NPAIR):
                        nc.tensor.matmul(
                            ps_q[:],
                            wq_sb[:, j, :, 128 * h : 128 * h + 128],
                            x8_t[:, j],
                            start=(j == 0), stop=(j == NPAIR - 1), perf_mode=DR,
                        )
                    qrope = qro.tile([128, 512], BF16, tag="qro")
                    rope(qrope[:], ps_q[:], tcs)
                    qropes.append(qrope)
                    sq = sml.tile([128, 512], BF16, tag="sq")
                    nc.vector.tensor_mul(sq[:], qrope[:], qrope[:])
                    for jj in range(4):
                        nc.tensor.matmul(
                            ps_qc[:, 4 * h + jj : 4 * h + jj + 1],
                            sq[:, 128 * jj : 128 * jj + 128],
                            ones_bf[:],
                            start=True, stop=True,
                        )
                u_q = sml.tile([128, 16], F32, tag="uq")
                nc.scalar.activation(
                    out=u_q[:], in_=ps_qc[:], func=AF.Ln, scale=1.0 / HD,
                    bias=eps_c[:],
                )
                rqc = sml.tile([128, 16], BF16, tag="rqc")
                nc.scalar.activation(
                    out=rqc[:], in_=u_q[:], func=AF.Exp, scale=-0.5, bias=nlhd_c[:]
                )
                # transpose the 16 norm columns to rows, then linearize the
                # rows onto partition 0 so partition_broadcast can read them:
                # rq_row[0, 128*(4h+jj)+t'] = 1/(HD*rms_q) of head h, block jj
                ps_rq = ps.tile([16, 128], BF16, tag="ps")
                nc.tensor.transpose(ps_rq[:], rqc[:], ident_bf[:])
                rq16 = sml.tile([16, 128], BF16, tag="rqs")
                nc.vector.tensor_copy(out=rq16[:], in_=ps_rq[:])
                # partition-flattening SBUF->SBUF DMA fails NEFF load;
                # bounce through DRAM where the flatten is just addressing
                drq = dram.tile([16, 128], BF16, tag="drq")
                nc.sync.dma_start(out=drq[:], in_=rq16[:])
                rq_row = sml.tile([1, 2048], BF16, tag="rqrow")
                nc.sync.dma_start(
                    out=rq_row[:], in_=drq[:].rearrange("p m -> (p m)")
                )
                for h in range(G):
                    bcq = sml.tile([128, 512], BF16, tag="bcq")
                    nc.gpsimd.partition_broadcast(
                        bcq[:], rq_row[0:1, 512 * h : 512 * h + 512]
                    )
                    nc.vector.tensor_mul(qT_sb[:, h, tcs], qropes[h][:], bcq[:])

            # ---- phase 2+4, tqc outer: attention, out-proj, ReduceScatter ----
            for tqc in range(NTCH):
                tqs = slice(512 * tqc, 512 * tqc + 512)
                nblk = 4 * tqc + 4
                for h in range(G):
                    ps_y = ps.tile([128, 512], F32, tag="ps", name=f"psy{tqc}_{h}")
                    ps_rs = ps.tile([1, 512], F32, tag="ps", name=f"psr{tqc}_{h}")
                    for tkb in range(4 * tqc):  # full blocks
                        ps_s = ps.tile([128, 512], F32, tag="ps")
                        nc.tensor.matmul(
                            ps_s[:],
                            kT_sb[:, 128 * tkb : 128 * tkb + 128],
                            qT_sb[:, h, tqs],
                            start=True, stop=True,
                        )
                        pT = pts.tile([128, 512], BF16, tag="pt")
                        nc.scalar.activation(
                            out=pT[:], in_=ps_s[:], func=AF.Exp,
                            scale=rk_col[:, tkb : tkb + 1],
                        )
                        nc.tensor.matmul(
                            ps_rs[:], ones_bf[:], pT[:],
                            start=(tkb == 0), stop=False,
                        )
                        nc.tensor.matmul(
                            ps_y[:], v_sb[:, tkb, :], pT[:],
                            start=(tkb == 0), stop=False,
                        )
                    for dd in range(4):  # diagonal blocks, trimmed
                        tkb = 4 * tqc + dd
                        w = 512 - 128 * dd
                        qs = slice(512 * tqc + 128 * dd, 512 * tqc + 512)
                        ps_s = ps.tile([128, 512], F32, tag="ps")
                        nc.tensor.matmul(
                            ps_s[:, 0:w],
                            kT_sb[:, 128 * tkb : 128 * tkb + 128],
                            qT_sb[:, h, qs],
                            start=True, stop=True,
                        )
                        pT = pts.tile([128, 512], BF16, tag="pt")
                        nc.scalar.activation(
                            out=pT[:, 0:w], in_=ps_s[:, 0:w], func=AF.Exp,
                            scale=rk_col[:, tkb : tkb + 1],
                        )
                        nc.vector.tensor_mul(
                            pT[:, 0:w], pT[:, 0:w], mask_sb[:, 0:w]
                        )
                        nc.tensor.matmul(
                            ps_rs[:, 128 * dd : 512], ones_bf[:], pT[:, 0:w],
                            start=(tkb == 0), stop=(dd == 3),
                        )
                        nc.tensor.matmul(
                            ps_y[:, 128 * dd : 512], v_sb[:, tkb, :], pT[:, 0:w],
                            start=(tkb == 0), stop=(dd == 3),
                        )
                    rrow = sml.tile([1, 512], F32, tag="rrow")
                    nc.vector.reciprocal_approx_fast(out=rrow[:], in_=ps_rs[:])
                    bc = sml.tile([128, 512], F32, tag="bc")
                    nc.gpsimd.partition_broadcast(bc[:], rrow[:])
                    nc.vector.tensor_mul(yT_sb[:, h, tqs], ps_y[:], bc[:])

                # ---- partial out-proj for this t-chunk + ReduceScatter ----
                partial = dram.tile([C, 512], BF16, tag="partial")
                for cob in range(NCB):
                    ps_o = ps.tile([128, 512], F32, tag="ps")
                    for h2 in range(G):
                        nc.tensor.matmul(
                            ps_o[:],
                            wp_sb[:, h2, 128 * cob : 128 * cob + 128],
                            yT_sb[:, h2, tqs],
                            start=(h2 == 0), stop=(h2 == G - 1),
                        )
                    po = pos.tile([128, 512], BF16, tag="po")
                    nc.vector.tensor_copy(out=po[:], in_=ps_o[:])
                    nc.sync.dma_start(
                        out=partial[128 * cob : 128 * cob + 128, :], in_=po[:]
                    )
                rs_out = dram.tile([512, 512], BF16, tag="rsout")
                nc.gpsimd.collective_compute(
                    "ReduceScatter",
                    mybir.AluOpType.add,
                    replica_groups=groups,
                    ins=[partial[:]],
                    outs=[rs_out[:]],
                )
                nc.sync.dma_start(out=outT[tqc], in_=rs_out[:])

    nc.compile()
    return nc


def _get_nc():
    if "nc" not in _CACHE:
        _CACHE["nc"] = _build()
    return _CACHE["nc"]


def _prep_core_inputs(x, cos, sin, Wq, Wk, Wv, Wp):
    f32 = np.float32
    bf16 = ml_dtypes.bfloat16
    f8 = ml_dtypes.float8_e4m3
    cosT = np.asarray(cos, dtype=f32).T  # [64, T]
    sinT = np.asarray(sin, dtype=f32).T
    cos2 = np.ascontiguousarray(np.vstack([cosT, cosT])).astype(bf16)
    sin2 = np.ascontiguousarray(np.vstack([-sinT, sinT])).astype(bf16)
    p = np.arange(128)[:, None]
    j = np.arange(512)[None, :]
    maskin = (j >= p).astype(bf16)

    in_maps = []
    for core in range(8):
        b, g = core // 4, core % 4
        xT = np.ascontiguousarray(np.asarray(x)[b].T).astype(f32)  # [C, T]
        x8 = np.ascontiguousarray(
            xT.reshape(NPAIR, 2, 128, T).transpose(2, 0, 1, 3)
        ).astype(f8)
        xbm = np.ascontiguousarray(
            xT.reshape(NCB, 128, T).transpose(1, 0, 2)
        ).astype(bf16)
        wq8 = np.ascontiguousarray(
            (np.asarray(Wq)[:, 512 * g : 512 * g + 512] * 64.0)
            .reshape(NPAIR, 2, 128, 512)
            .transpose(2, 0, 1, 3)
        ).astype(f8)
        wk8 = np.ascontiguousarray(
            (np.asarray(Wk)[:, 128 * g : 128 * g + 128] * 64.0)
            .reshape(NPAIR, 2, 128, 128)
            .transpose(2, 0, 1, 3)
        ).astype(f8)
        wvm = np.ascontiguousarray(
            np.asarray(Wv)[:, 128 * g : 128 * g + 128]
            .reshape(NCB, 128, 128)
            .transpose(1, 0, 2)
        ).astype(bf16)
        wpm = np.ascontiguousarray(
            np.asarray(Wp)[512 * g : 512 * g + 512, :]
            .reshape(G, 128, C)
            .transpose(1, 0, 2)
        ).astype(bf16)
        in_maps.append(
            {
                "x8": x8, "xb": xbm, "wq8": wq8, "wk8": wk8, "wv": wvm,
                "wp": wpm, "cos2": cos2, "sin2": sin2, "maskin": maskin,
            }
        )
    return in_maps


def kernel(x, cos, sin, Wq, Wk, Wv, Wp):
    from concourse.bass_utils import run_bass_kernel_spmd

    in_maps = _prep_core_inputs(x, cos, sin, Wq, Wk, Wv, Wp)
    nc = _get_nc()
    res = run_bass_kernel_spmd(nc, in_maps, core_ids=list(range(8)), trace=False)

    out = np.empty((B, T, C), dtype=np.float32)
    for core in range(8):
        b, g = core // 4, core % 4
        o = np.asarray(res.results[core]["outT"], dtype=np.float32)  # [4,512,512]
        for tch in range(NTCH):
            out[b, 512 * tch : 512 * tch + 512, 512 * g : 512 * g + 512] = o[tch].T
    return out
